# revision 7
# baseline (speedup 1.0000x reference)
"""StyleGAN-style modulated 3x3 conv on 8 Trainium2 NeuronCores.

Problem: y = conv2d(x, kernel * (style+1) / demod), SAME padding,
  x [B=8, H=128, W=128, C=256], kernel [3,3,C=256,F=256],
  style [B,1,1,C], demod[f] = sqrt(sum_{ky,kx,c} wmod^2 + 1e-8).

Sharding: data-parallel over batch B — each of the 8 cores convolves one
sample with its own modulated/demodulated kernel. No cross-core comm.

Algorithm (per core): 1D Winograd F(2,3) along H + direct 3-tap conv
along W, all matmuls in bf16 (PE full rate; rel-err gate is 2e-2, bf16
lands ~3e-3). This cuts PE work to 2/3 of the direct implicit-GEMM
floor: per output row pair, 4 transformed planes replace 6 tap rows.

  - input transform (DVE, bf16 2x): T0=d0-d2, T1=d1+d2, T2=d2-d1,
    T3=d1-d3 where d_k = x_pad[c, 2*ht+k, :] (rows on the free axis,
    full-rate unit-stride inner dim). x streamed in 10-row slots
    (8 new + 2 halo) so x never fully resides in SBUF.
  - weight transform (DVE, bf16): W0=m0, W1=(m0+m1+m2)/2,
    W2=(m0-m1+m2)/2, W3=m2 with m_ky = raw[ky]*(style+1).
  - GEMM (PE): M_j[f,p] += sum_{kx,c} T_j[c, p+kx-1] * W_j[kx,c,f],
    PSUM tile [f=128, 512 px], 24 bf16 matmuls per (group, f-half).
  - output transform fused with demodulation (DVE scalar_tensor_tensor
    with per-partition scalar invd[f]):
      y_even = (M0+M1+M2)*invd, y_odd = (M1-M2-M3)*invd
    via c1s = M1*invd (ACT copy w/ scale), then 4 STT ops; outputs
    stored bf16 (halves store traffic), upcast on host.
  - demod invd from bf16 raw weights: ACT Square(raw*s) then the
    ones-column matmul trick per f-half; emitted after conv group 0
    so it does not gate the conv start.

Host does layout-only marshalling: shard over B, transpose+zero-pad x
to [C, 130, 130] bf16 per core; reshape kernel to [CH,128,9,F] bf16;
un-interleave parity planes + strip pad columns on gather.
"""

import sys
import os

for _p in ("/opt/trn_rl_repo", "/root/.axon_site", "/root/.axon_site/_ro/trn_rl_repo",
           "/root/.axon_site/_ro/pypackages"):
    if os.path.isdir(_p) and _p not in sys.path:
        sys.path.append(_p)

import numpy as np
import ml_dtypes

B, H, W, C, F = 8, 128, 128, 256, 256
CH = C // 128                  # contraction halves
FHN = F // 128                 # f halves
NTAP = 9
WW = W + 2                     # padded width  (w = -1..128)
HP = H + 2                     # padded height (h = -1..128)
NHT = H // 2                   # 64 h-tiles (output row pairs)
TLEN = NHT * WW                # 8320 flat transformed positions
GT = 8                         # zero guard around T planes (+-1 shifts)
NSLOT = 16                     # x slots: 8 fresh rows + 2 halo rows each
SROWS = 10
CHT = 4                        # h-tiles transformed per chunk (= 1 slot)
CLEN = CHT * WW                # 520
NG = 17                        # PE groups per f-half: 16 x 512 + 1 x 128
N_CORES = 8

_COMPILED = {}


def _build_nc():
    import concourse.bacc as bacc
    import concourse.mybir as mybir
    import concourse.tile as tile

    f32 = mybir.dt.float32
    bf16 = mybir.dt.bfloat16
    AF = mybir.ActivationFunctionType
    ALU = mybir.AluOpType

    nc = bacc.Bacc("TRN2", target_bir_lowering=False, debug=False,
                   num_devices=N_CORES)

    xt_d = nc.dram_tensor("xt", [CH, 128, HP * WW], bf16,
                          kind="ExternalInput").ap()
    st_d = nc.dram_tensor("st", [128, CH], f32, kind="ExternalInput").ap()
    wk_d = nc.dram_tensor("wk", [CH, 128, NTAP, F], bf16,
                          kind="ExternalInput").ap()
    # yt[parity][f_half][f][flat ht*WW+w+1]; pad cols stripped on host
    yt_d = nc.dram_tensor("yt", [2, FHN, 128, TLEN], bf16,
                          kind="ExternalOutput").ap()

    with tile.TileContext(nc) as tc:
        with tc.tile_pool(name="pers", bufs=1) as pers, \
             tc.tile_pool(name="xs", bufs=3) as xs, \
             tc.tile_pool(name="wtmp", bufs=1) as wtmp, \
             tc.tile_pool(name="dtmp", bufs=2) as dtmp, \
             tc.tile_pool(name="stage", bufs=3) as stage, \
             tc.tile_pool(name="ps", bufs=7, space="PSUM") as ps, \
             tc.tile_pool(name="psd", bufs=1, space="PSUM") as psd:

            # ---- style scalars ----
            s_t = pers.tile([128, CH], f32, tag="s", name="s_t")
            nc.sync.dma_start(s_t[:], st_d)
            nc.vector.tensor_scalar_add(s_t[:], s_t[:], 1.0)
            hs_t = pers.tile([128, CH], f32, tag="hs", name="hs_t")
            nc.vector.tensor_scalar_mul(hs_t[:], s_t[:], 0.5)

            # ---- T planes: [c, GT + 8320 + GT] bf16, guards zeroed ----
            tp = [[pers.tile([128, GT + TLEN + GT], bf16, tag=f"T{j}_{ch}",
                             name=f"T{j}_{ch}") for ch in range(CH)]
                  for j in range(4)]
            for j in range(4):
                for ch in range(CH):
                    nc.vector.memset(tp[j][ch][:, 0:GT], 0.0)
                    nc.vector.memset(tp[j][ch][:, GT + TLEN:], 0.0)

            # ---- x slot DMA + input transform emission helpers ----
            slot_tiles = {}

            def emit_slot_dma(s):
                if s >= NSLOT or s in slot_tiles:
                    return
                tl = []
                for ch in range(CH):
                    t = xs.tile([128, SROWS, WW], bf16, tag=f"x{ch}",
                                name=f"x{s}_{ch}")
                    nc.sync.dma_start(
                        t[:], xt_d[ch][:, 8 * s * WW:(8 * s + SROWS) * WW])
                    tl.append(t)
                slot_tiles[s] = tl

            done_chunks = set()

            def emit_transform(c):
                if c >= NSLOT or c in done_chunks:
                    return
                done_chunks.add(c)
                emit_slot_dma(c + 3)
                for ch in range(CH):
                    sl = slot_tiles[c][ch]
                    d = [sl[:, k:k + 2 * CHT - 1:2, :] for k in range(4)]
                    o = [tp[j][ch][:, GT + CLEN * c:GT + CLEN * (c + 1)]
                         .rearrange("p (a b) -> p a b", a=CHT)
                         for j in range(4)]
                    nc.vector.tensor_sub(o[0], d[0], d[2])
                    nc.vector.tensor_add(o[1], d[1], d[2])
                    nc.vector.tensor_sub(o[2], d[2], d[1])
                    nc.vector.tensor_sub(o[3], d[1], d[3])

            # ---- raw weights (bf16): DMA order interleaved with x slots
            # so both the first weight combos (need ky0) and the first
            # transform chunk (needs slot0) are ready asap ----
            wraw = [pers.tile([128, NTAP, F], bf16, tag=f"wraw{ch}",
                              name=f"wraw{ch}") for ch in range(CH)]

            def dma_wk(ky):
                for ch in range(CH):
                    nc.sync.dma_start(wraw[ch][:, 3 * ky:3 * ky + 3],
                                      wk_d[ch][:, 3 * ky:3 * ky + 3])

            dma_wk(0)
            emit_slot_dma(0)
            emit_slot_dma(1)
            dma_wk(2)
            emit_slot_dma(2)
            dma_wk(1)

            # ---- weight transform: Wt[j][ch] [c, kx, f] bf16, staged so
            # j0 (first matmuls) is ready earliest; transforms interleave ----
            wt = [[pers.tile([128, 3, F], bf16, tag=f"wt{j}_{ch}",
                             name=f"wt{j}_{ch}") for ch in range(CH)]
                  for j in range(4)]
            for ch in range(CH):
                nc.vector.tensor_scalar_mul(wt[0][ch][:], wraw[ch][:, 0:3],
                                            s_t[:, ch:ch + 1])
            emit_transform(0)
            for ch in range(CH):
                nc.vector.tensor_scalar_mul(wt[3][ch][:], wraw[ch][:, 6:9],
                                            s_t[:, ch:ch + 1])
            emit_transform(1)
            for ch in range(CH):
                su = wtmp.tile([128, 3, F], bf16, tag="su", name="su")
                nc.vector.tensor_add(su[:], wraw[ch][:, 0:3], wraw[ch][:, 6:9])
                sv = wtmp.tile([128, 3, F], bf16, tag="sv", name="sv")
                nc.vector.tensor_add(sv[:], su[:], wraw[ch][:, 3:6])
                nc.vector.tensor_scalar_mul(wt[1][ch][:], sv[:],
                                            hs_t[:, ch:ch + 1])
                sw = wtmp.tile([128, 3, F], bf16, tag="sw", name="sw")
                nc.vector.tensor_sub(sw[:], su[:], wraw[ch][:, 3:6])
                nc.vector.tensor_scalar_mul(wt[2][ch][:], sw[:],
                                            hs_t[:, ch:ch + 1])

            # ---- demod inputs: sq = (raw*s)^2, bf16 (ACT) ----
            sq = [pers.tile([128, NTAP, F], bf16, tag=f"sq{ch}",
                            name=f"sq{ch}") for ch in range(CH)]
            for ch in range(CH):
                nc.scalar.activation(sq[ch][:], wraw[ch][:], AF.Square,
                                     scale=s_t[:, ch:ch + 1])
            ones_t = pers.tile([128, 1], bf16, tag="ones", name="ones_t")
            nc.vector.memset(ones_t[:], 1.0)
            eps_t = pers.tile([128, 1], f32, tag="eps", name="eps_t")
            nc.vector.memset(eps_t[:], 1e-8)
            iv = [pers.tile([128, 1], f32, tag=f"iv{fh}", name=f"iv{fh}")
                  for fh in range(FHN)]

            # ---- main loop ----
            JORD = (0, 3, 1, 2)      # j0/j3 weights are ready earliest
            for g in range(NG):
                npx = 512 if g < NG - 1 else TLEN - 512 * (NG - 1)
                need = min(NSLOT - 1, (512 * (g + 1)) // CLEN)
                for c in range(need + 1):
                    emit_transform(c)

                mt = {}
                for fh in range(FHN):
                    for j in JORD:
                        m = ps.tile([128, 512], f32, tag="m",
                                    name=f"m{j}_{g}_{fh}")
                        mt[j] = m
                        i = 0
                        for kx in range(3):
                            for ch in range(CH):
                                rhs = tp[j][ch][:, GT + 512 * g + kx - 1:
                                                GT + 512 * g + kx - 1 + npx]
                                nc.tensor.matmul(
                                    m[:, :npx],
                                    wt[j][ch][:, kx, fh * 128:(fh + 1) * 128],
                                    rhs, start=(i == 0), stop=(i == 5))
                                i += 1

                    if g == 0 and fh == 0:
                        # demod: d2[f] = sum taps/c of sq; ones-matmul trick.
                        # Emitted after group 0's matmuls: PE stays busy and
                        # invd is ready exactly when group 0 drains.
                        for dfh in range(FHN):
                            d2 = psd.tile([128, 1], f32, tag="d2",
                                          name=f"d2_{dfh}")
                            i = 0
                            for ch in range(CH):
                                for t in range(NTAP):
                                    nc.tensor.matmul(
                                        d2[:],
                                        sq[ch][:, t, dfh * 128:(dfh + 1) * 128],
                                        ones_t[:], start=(i == 0),
                                        stop=(i == CH * NTAP - 1))
                                    i += 1
                            dm = dtmp.tile([128, 1], f32, tag="dm", name="dm")
                            nc.scalar.activation(dm[:], d2[:], AF.Sqrt,
                                                 bias=eps_t[:])
                            nc.vector.reciprocal(iv[dfh][:], dm[:])

                    # ---- drain: ACT scales each M_j by invd into bf16
                    # SBUF planes (ACT is otherwise idle), DVE combines at
                    # bf16 2x rate: y_e=(c0+c1)+c2, y_o=(c1-c2)-c3 ----
                    cs = []
                    for j in range(4):
                        cj = dtmp.tile([128, 512], bf16, tag=f"c{j}",
                                       name=f"c{j}")
                        nc.scalar.activation(cj[:, :npx], mt[j][:, :npx],
                                             AF.Copy, scale=iv[fh][:])
                        cs.append(cj)
                    te = dtmp.tile([128, 512], bf16, tag="te", name="te")
                    nc.vector.tensor_add(te[:, :npx], cs[0][:, :npx],
                                         cs[1][:, :npx])
                    oe = stage.tile([128, 512], bf16, tag="oe", name="oe")
                    nc.vector.tensor_add(oe[:, :npx], te[:, :npx],
                                         cs[2][:, :npx])
                    to = dtmp.tile([128, 512], bf16, tag="to", name="to")
                    nc.vector.tensor_sub(to[:, :npx], cs[1][:, :npx],
                                         cs[2][:, :npx])
                    oo = stage.tile([128, 512], bf16, tag="oo", name="oo")
                    nc.vector.tensor_sub(oo[:, :npx], to[:, :npx],
                                         cs[3][:, :npx])
                    nc.gpsimd.dma_start(
                        yt_d[0][fh][:, 512 * g:512 * g + npx], oe[:, :npx])
                    nc.gpsimd.dma_start(
                        yt_d[1][fh][:, 512 * g:512 * g + npx], oo[:, :npx])

    nc.compile()
    return nc


def _get_nc():
    if "nc" not in _COMPILED:
        _COMPILED["nc"] = _build_nc()
    return _COMPILED["nc"]


def _prep_in_maps(x, style, kernel):
    """Host-side layout marshalling: shard over B, transpose+pad+cast x."""
    bf = ml_dtypes.bfloat16
    x = np.ascontiguousarray(x, dtype=np.float32)
    style = np.ascontiguousarray(style, dtype=np.float32)
    kernel = np.ascontiguousarray(kernel, dtype=np.float32)
    # [3,3,C,F] -> [c_half, c_low, tap, f], bf16
    wk = np.ascontiguousarray(
        kernel.reshape(NTAP, CH, 128, F).transpose(1, 2, 0, 3)).astype(bf)
    in_maps = []
    for b in range(B):
        xp = np.zeros((C, HP, WW), dtype=np.float32)
        xp[:, 1:H + 1, 1:W + 1] = x[b].transpose(2, 0, 1)
        xt = np.ascontiguousarray(
            xp.reshape(CH, 128, HP * WW)).astype(bf)
        st = np.ascontiguousarray(style[b].reshape(CH, 128).T)
        in_maps.append({"xt": xt, "st": st, "wk": wk})
    return in_maps


def run_cores(x, style, kernel, trace=False, trace_cores=None):
    """Compile (cached) + run on the 8 NeuronCores. Returns (y, results)."""
    from concourse.bass_utils import run_bass_kernel_spmd

    nc = _get_nc()
    in_maps = _prep_in_maps(x, style, kernel)
    kwargs = {}
    if trace:
        kwargs.update(trace=True, trace_cores=trace_cores)
    res = run_bass_kernel_spmd(nc, in_maps, list(range(N_CORES)), **kwargs)
    y = np.empty((B, H, W, F), dtype=np.float32)
    for b in range(B):
        yt = np.asarray(res.results[b]["yt"]).astype(np.float32)
        # [2, FHN, 128, TLEN] -> strip pad cols, interleave parity rows
        for p in range(2):
            for fh in range(FHN):
                pl = yt[p, fh].reshape(128, NHT, WW)[:, :, 1:W + 1]
                y[b, p::2, :, fh * 128:(fh + 1) * 128] = pl.transpose(1, 2, 0)
    return y, res


def kernel(x, style, kernel):
    y, _ = run_cores(x, style, kernel)
    return y.astype(np.float32)


# revision 13
# speedup vs baseline: 1.0159x; 1.0159x over previous
"""StyleGAN-style modulated 3x3 conv on 8 Trainium2 NeuronCores.

Problem: y = conv2d(x, kernel * (style+1) / demod), SAME padding,
  x [B=8, H=128, W=128, C=256], kernel [3,3,C=256,F=256],
  style [B,1,1,C], demod[f] = sqrt(sum_{ky,kx,c} wmod^2 + 1e-8).

Sharding: data-parallel over batch B — each of the 8 cores convolves one
sample with its own modulated/demodulated kernel. No cross-core comm.

Algorithm (per core): 1D Winograd F(2,3) along H + direct 3-tap conv
along W, all matmuls in bf16 (PE full rate; rel-err gate is 2e-2, bf16
lands ~3e-3). This cuts PE work to 2/3 of the direct implicit-GEMM
floor: per output row pair, 4 transformed planes replace 6 tap rows.

  - input transform (DVE, bf16 2x): T0=d0-d2, T1=d1+d2, T2=d2-d1,
    T3=d1-d3 where d_k = x_pad[c, 2*ht+k, :] (rows on the free axis,
    full-rate unit-stride inner dim). x streamed in 10-row slots
    (8 new + 2 halo) so x never fully resides in SBUF.
  - weight transform (DVE, bf16): W0=m0, W1=(m0+m1+m2)/2,
    W2=(m0-m1+m2)/2, W3=m2 with m_ky = raw[ky]*(style+1).
  - GEMM (PE): M_j[f,p] += sum_{kx,c} T_j[c, p+kx-1] * W_j[kx,c,f],
    PSUM tile [f=128, 512 px], 24 bf16 matmuls per (group, f-half).
  - output transform fused with demodulation (DVE scalar_tensor_tensor
    with per-partition scalar invd[f]):
      y_even = (M0+M1+M2)*invd, y_odd = (M1-M2-M3)*invd
    via c1s = M1*invd (ACT copy w/ scale), then 4 STT ops; outputs
    stored bf16 (halves store traffic), upcast on host.
  - demod invd from bf16 raw weights: ACT Square(raw*s) then the
    ones-column matmul trick per f-half; emitted after conv group 0
    so it does not gate the conv start.

Host does layout-only marshalling: shard over B, transpose+zero-pad x
to [C, 130, 130] bf16 per core; reshape kernel to [CH,128,9,F] bf16;
un-interleave parity planes + strip pad columns on gather.
"""

import sys
import os

for _p in ("/opt/trn_rl_repo", "/root/.axon_site", "/root/.axon_site/_ro/trn_rl_repo",
           "/root/.axon_site/_ro/pypackages"):
    if os.path.isdir(_p) and _p not in sys.path:
        sys.path.append(_p)

import numpy as np
import ml_dtypes

B, H, W, C, F = 8, 128, 128, 256, 256
CH = C // 128                  # contraction halves
FHN = F // 128                 # f halves
NTAP = 9
WW = W + 2                     # padded width  (w = -1..128)
HP = H + 2                     # padded height (h = -1..128)
NHT = H // 2                   # 64 h-tiles (output row pairs)
TLEN = NHT * WW                # 8320 flat transformed positions
GT = 8                         # zero guard around T planes (+-1 shifts)
NSLOT = 16                     # x slots: 8 fresh rows + 2 halo rows each
SROWS = 10
CHT = 4                        # h-tiles transformed per chunk (= 1 slot)
CLEN = CHT * WW                # 520
NG = 17                        # PE groups per f-half: 16 x 512 + 1 x 128
N_CORES = 8

_COMPILED = {}


def _build_nc():
    import concourse.bacc as bacc
    import concourse.mybir as mybir
    import concourse.tile as tile

    f32 = mybir.dt.float32
    bf16 = mybir.dt.bfloat16
    AF = mybir.ActivationFunctionType
    ALU = mybir.AluOpType

    nc = bacc.Bacc("TRN2", target_bir_lowering=False, debug=False,
                   num_devices=N_CORES)

    xt_d = nc.dram_tensor("xt", [CH, 128, HP * WW], bf16,
                          kind="ExternalInput").ap()
    st_d = nc.dram_tensor("st", [128, CH], f32, kind="ExternalInput").ap()
    wk_d = nc.dram_tensor("wk", [CH, 128, NTAP, F], bf16,
                          kind="ExternalInput").ap()
    # yt[f_half][f][parity][flat ht*WW+w+1]; pad cols stripped on host
    yt_d = nc.dram_tensor("yt", [FHN, 128, 2, TLEN], bf16,
                          kind="ExternalOutput").ap()

    with tile.TileContext(nc) as tc:
        with tc.tile_pool(name="pers", bufs=1) as pers, \
             tc.tile_pool(name="xs", bufs=3) as xs, \
             tc.tile_pool(name="wtmp", bufs=1) as wtmp, \
             tc.tile_pool(name="dtmp", bufs=2) as dtmp, \
             tc.tile_pool(name="stage", bufs=3) as stage, \
             tc.tile_pool(name="ps", bufs=7, space="PSUM") as ps, \
             tc.tile_pool(name="psd", bufs=1, space="PSUM") as psd:

            # ---- style scalars ----
            s_t = pers.tile([128, CH], f32, tag="s", name="s_t")
            nc.sync.dma_start(s_t[:], st_d)
            nc.vector.tensor_scalar_add(s_t[:], s_t[:], 1.0)
            hs_t = pers.tile([128, CH], f32, tag="hs", name="hs_t")
            nc.vector.tensor_scalar_mul(hs_t[:], s_t[:], 0.5)

            # ---- T planes: [c, GT + 8320 + GT] bf16, guards zeroed ----
            tp = [[pers.tile([128, GT + TLEN + GT], bf16, tag=f"T{j}_{ch}",
                             name=f"T{j}_{ch}") for ch in range(CH)]
                  for j in range(4)]
            for j in range(4):
                for ch in range(CH):
                    nc.vector.memset(tp[j][ch][:, 0:GT], 0.0)
                    nc.vector.memset(tp[j][ch][:, GT + TLEN:], 0.0)

            # ---- x slot DMA + input transform emission helpers ----
            slot_tiles = {}

            def emit_slot_dma(s):
                if s >= NSLOT or s in slot_tiles:
                    return
                tl = []
                for ch in range(CH):
                    t = xs.tile([128, SROWS, WW], bf16, tag=f"x{ch}",
                                name=f"x{s}_{ch}")
                    eng = nc.scalar if ch == 0 else nc.gpsimd
                    eng.dma_start(
                        t[:], xt_d[ch][:, 8 * s * WW:(8 * s + SROWS) * WW])
                    tl.append(t)
                slot_tiles[s] = tl

            done_chunks = set()

            def emit_transform(c):
                if c >= NSLOT or c in done_chunks:
                    return
                done_chunks.add(c)
                emit_slot_dma(c + 3)
                for ch in range(CH):
                    sl = slot_tiles[c][ch]
                    d = [sl[:, k:k + 2 * CHT - 1:2, :] for k in range(4)]
                    o = [tp[j][ch][:, GT + CLEN * c:GT + CLEN * (c + 1)]
                         .rearrange("p (a b) -> p a b", a=CHT)
                         for j in range(4)]
                    nc.vector.tensor_sub(o[0], d[0], d[2])
                    nc.vector.tensor_add(o[1], d[1], d[2])
                    nc.vector.tensor_sub(o[2], d[2], d[1])
                    nc.vector.tensor_sub(o[3], d[1], d[3])

            # ---- raw weights (bf16): DMA order interleaved with x slots
            # so both the first weight combos (need ky0) and the first
            # transform chunk (needs slot0) are ready asap ----
            wraw = [pers.tile([128, NTAP, F], bf16, tag=f"wraw{ch}",
                              name=f"wraw{ch}") for ch in range(CH)]

            def dma_wk(ky):
                for ch in range(CH):
                    nc.sync.dma_start(wraw[ch][:, 3 * ky:3 * ky + 3],
                                      wk_d[ch][:, 3 * ky:3 * ky + 3])

            dma_wk(0)
            emit_slot_dma(0)
            emit_slot_dma(1)
            dma_wk(2)
            emit_slot_dma(2)
            dma_wk(1)

            # ---- weight transform: Wt[j][ch] [c, kx, f] bf16, staged so
            # j0 (first matmuls) is ready earliest; transforms interleave ----
            wt = [[pers.tile([128, 3, F], bf16, tag=f"wt{j}_{ch}",
                             name=f"wt{j}_{ch}") for ch in range(CH)]
                  for j in range(4)]
            for ch in range(CH):
                nc.vector.tensor_scalar_mul(wt[0][ch][:], wraw[ch][:, 0:3],
                                            s_t[:, ch:ch + 1])
            emit_transform(0)
            for ch in range(CH):
                nc.vector.tensor_scalar_mul(wt[3][ch][:], wraw[ch][:, 6:9],
                                            s_t[:, ch:ch + 1])
            emit_transform(1)
            for ch in range(CH):
                su = wtmp.tile([128, 3, F], bf16, tag="su", name="su")
                nc.vector.tensor_add(su[:], wraw[ch][:, 0:3], wraw[ch][:, 6:9])
                sv = wtmp.tile([128, 3, F], bf16, tag="sv", name="sv")
                nc.vector.tensor_add(sv[:], su[:], wraw[ch][:, 3:6])
                nc.vector.tensor_scalar_mul(wt[1][ch][:], sv[:],
                                            hs_t[:, ch:ch + 1])
                sw = wtmp.tile([128, 3, F], bf16, tag="sw", name="sw")
                nc.vector.tensor_sub(sw[:], su[:], wraw[ch][:, 3:6])
                nc.vector.tensor_scalar_mul(wt[2][ch][:], sw[:],
                                            hs_t[:, ch:ch + 1])

            # ---- demod inputs: sq = (raw*s)^2, bf16 (ACT) ----
            sq = [pers.tile([128, NTAP, F], bf16, tag=f"sq{ch}",
                            name=f"sq{ch}") for ch in range(CH)]
            for ch in range(CH):
                nc.scalar.activation(sq[ch][:], wraw[ch][:], AF.Square,
                                     scale=s_t[:, ch:ch + 1])
            ones_t = pers.tile([128, 1], bf16, tag="ones", name="ones_t")
            nc.vector.memset(ones_t[:], 1.0)
            eps_t = pers.tile([128, 1], f32, tag="eps", name="eps_t")
            nc.vector.memset(eps_t[:], 1e-8)
            iv = [pers.tile([128, 1], f32, tag=f"iv{fh}", name=f"iv{fh}")
                  for fh in range(FHN)]

            # ---- main loop ----
            JORD = (0, 3, 1, 2)      # j0/j3 weights are ready earliest
            for g in range(NG):
                npx = 512 if g < NG - 1 else TLEN - 512 * (NG - 1)
                need = min(NSLOT - 1, (512 * (g + 3)) // CLEN)
                for c in range(need + 1):
                    emit_transform(c)

                mt = {}
                for fh in range(FHN):
                    for j in JORD:
                        m = ps.tile([128, 512], f32, tag="m",
                                    name=f"m{j}_{g}_{fh}")
                        mt[j] = m
                        i = 0
                        for kx in range(3):
                            for ch in range(CH):
                                rhs = tp[j][ch][:, GT + 512 * g + kx - 1:
                                                GT + 512 * g + kx - 1 + npx]
                                nc.tensor.matmul(
                                    m[:, :npx],
                                    wt[j][ch][:, kx, fh * 128:(fh + 1) * 128],
                                    rhs, start=(i == 0), stop=(i == 5))
                                i += 1

                    if g == 0 and fh == 0:
                        # demod: d2[f] = sum taps/c of sq; ones-matmul trick.
                        # Emitted after group 0's matmuls: PE stays busy and
                        # invd is ready exactly when group 0 drains.
                        for dfh in range(FHN):
                            d2 = psd.tile([128, 1], f32, tag="d2",
                                          name=f"d2_{dfh}")
                            i = 0
                            for ch in range(CH):
                                for t in range(NTAP):
                                    nc.tensor.matmul(
                                        d2[:],
                                        sq[ch][:, t, dfh * 128:(dfh + 1) * 128],
                                        ones_t[:], start=(i == 0),
                                        stop=(i == CH * NTAP - 1))
                                    i += 1
                            dm = dtmp.tile([128, 1], f32, tag="dm", name="dm")
                            nc.scalar.activation(dm[:], d2[:], AF.Sqrt,
                                                 bias=eps_t[:])
                            nc.vector.reciprocal(iv[dfh][:], dm[:])

                    # ---- drain: ACT scales each M_j by invd into bf16
                    # SBUF planes (ACT is otherwise idle), DVE combines at
                    # bf16 2x rate: y_e=(c0+c1)+c2, y_o=(c1-c2)-c3 ----
                    cs = []
                    for j in range(4):
                        cj = dtmp.tile([128, 512], bf16, tag=f"c{j}",
                                       name=f"c{j}")
                        nc.scalar.activation(cj[:, :npx], mt[j][:, :npx],
                                             AF.Copy, scale=iv[fh][:])
                        cs.append(cj)
                    te = dtmp.tile([128, 512], bf16, tag="te", name="te")
                    nc.vector.tensor_add(te[:, :npx], cs[0][:, :npx],
                                         cs[1][:, :npx])
                    to = dtmp.tile([128, 512], bf16, tag="to", name="to")
                    nc.vector.tensor_sub(to[:, :npx], cs[1][:, :npx],
                                         cs[2][:, :npx])
                    ob = stage.tile([128, 2, 512], bf16, tag="ob", name="ob")
                    nc.vector.tensor_add(ob[:, 0, :npx], te[:, :npx],
                                         cs[2][:, :npx])
                    nc.vector.tensor_sub(ob[:, 1, :npx], to[:, :npx],
                                         cs[3][:, :npx])
                    eng = nc.gpsimd if g % 2 == 0 else nc.sync
                    eng.dma_start(
                        yt_d[fh][:, :, 512 * g:512 * g + npx],
                        ob[:, :, :npx])

    nc.compile()
    return nc


def _get_nc():
    if "nc" not in _COMPILED:
        _COMPILED["nc"] = _build_nc()
    return _COMPILED["nc"]


def _prep_in_maps(x, style, kernel):
    """Host-side layout marshalling: shard over B, transpose+pad+cast x."""
    bf = ml_dtypes.bfloat16
    x = np.ascontiguousarray(x, dtype=np.float32)
    style = np.ascontiguousarray(style, dtype=np.float32)
    kernel = np.ascontiguousarray(kernel, dtype=np.float32)
    # [3,3,C,F] -> [c_half, c_low, tap, f], bf16
    wk = np.ascontiguousarray(
        kernel.reshape(NTAP, CH, 128, F).transpose(1, 2, 0, 3)).astype(bf)
    in_maps = []
    for b in range(B):
        xp = np.zeros((C, HP, WW), dtype=np.float32)
        xp[:, 1:H + 1, 1:W + 1] = x[b].transpose(2, 0, 1)
        xt = np.ascontiguousarray(
            xp.reshape(CH, 128, HP * WW)).astype(bf)
        st = np.ascontiguousarray(style[b].reshape(CH, 128).T)
        in_maps.append({"xt": xt, "st": st, "wk": wk})
    return in_maps


def run_cores(x, style, kernel, trace=False, trace_cores=None):
    """Compile (cached) + run on the 8 NeuronCores. Returns (y, results)."""
    from concourse.bass_utils import run_bass_kernel_spmd

    nc = _get_nc()
    in_maps = _prep_in_maps(x, style, kernel)
    kwargs = {}
    if trace:
        kwargs.update(trace=True, trace_cores=trace_cores)
    res = run_bass_kernel_spmd(nc, in_maps, list(range(N_CORES)), **kwargs)
    y = np.empty((B, H, W, F), dtype=np.float32)
    for b in range(B):
        yt = np.asarray(res.results[b]["yt"]).astype(np.float32)
        # [FHN, 128, 2, TLEN] -> strip pad cols, interleave parity rows
        for p in range(2):
            for fh in range(FHN):
                pl = yt[fh, :, p].reshape(128, NHT, WW)[:, :, 1:W + 1]
                y[b, p::2, :, fh * 128:(fh + 1) * 128] = pl.transpose(1, 2, 0)
    return y, res


def kernel(x, style, kernel):
    y, _ = run_cores(x, style, kernel)
    return y.astype(np.float32)
